# revision 5
# baseline (speedup 1.0000x reference)
"""GCN layer (gather -> mean-aggregate -> linear) on 8 Trainium2 cores.

Strategy (SPMD, no collectives):
  - Nodes are row-sharded: core c owns dst nodes [c*S, (c+1)*S), S = N/8.
  - Edges are bucketed by dst-owner core on the host and turned into a dense
    per-core adjacency count matrix A[src_node, local_dst] (fp8e4m3 - counts
    are small integers, exact). The per-core segment sum is computed
    FEATURE-MAJOR on the PE array:  sumsT = x.T @ A, with x as bf16 slabs
    (lhsT stationary) and A streaming as the rhs.  This streams R columns
    per src slab (vs 2*F+1 per slab per node-group for the node-major
    orientation) - half the PE cycles of the hi/lo node-major scheme.
  - Degrees (and the zero-in-degree fallback) are folded in on the host:
    rb = 1/max(deg,1) is shipped per node, and zero-degree nodes get a
    self-edge in A so mean==x for them (DGL recv semantics), making the
    device program branch-free.
  - Phase 1 runs in two column passes (cols 0:1024, then 1024:1250).  The
    second pass's matmul stream hides the drain of the first pass's eight
    node tiles: per tile, evacuate PSUM->SBUF (bf16), GEMM
    out = (sumsT_t.T @ W) (sumsT is already the lhsT layout - no PE
    transposes), fused scale(1/deg)+bias on DVE/GPSIMD, and the output DMA.
  - All input DMA is issued up front (A/x are fully SBUF-resident), split
    across the two HWDGE queues in consumption order.
"""

import os

import numpy as np

CORES = 8
TRACE = False           # set by test harness to print HW exec time
_cache = {}


def _build_program(N, F, FO, R, S):
    from concourse import bacc, tile
    from concourse.bass import mybir

    F32 = mybir.dt.float32
    BF16 = mybir.dt.bfloat16
    FP8 = mybir.dt.float8e4
    KT = (N + 127) // 128          # src-node slabs
    NT = (S + 127) // 128          # owned-node tiles per core
    nc = bacc.Bacc(None)

    xtd = nc.dram_tensor("xt", [128, KT * F], BF16, kind="ExternalInput")
    Ad = nc.dram_tensor("A", [128, KT * R], FP8, kind="ExternalInput")
    Wd = nc.dram_tensor("W", [F, FO], BF16, kind="ExternalInput")
    bd = nc.dram_tensor("b", [128, FO], F32, kind="ExternalInput")
    rbd = nc.dram_tensor("rb", [128, NT], F32, kind="ExternalInput")
    out = nc.dram_tensor("out", [R, FO], F32, kind="ExternalOutput")

    # PSUM bank map (each accumulating group owns a 2KB bank, zeroed by
    # its first start=True matmul):
    #   banks 0,1: pass-A sumsT col groups [0:512],[512:1024]
    #   bank  2:   pass-B col group [1024:S]
    #   banks 4,5,6: phase-3 out psum, cycling t%3
    #   bank  7 head: PE warm-up scratch
    psall = nc.alloc_psum_tensor("psall", [128, 4096], F32)

    passA = [(0, 512), (512, 1024)]
    passB = [(1024, S)]

    with tile.TileContext(nc) as tc:
        with (
            tc.tile_pool(name="const", bufs=1) as cpool,
            tc.tile_pool(name="acc", bufs=1) as accpool,
            tc.tile_pool(name="p3", bufs=4) as p3pool,
        ):
            x_sb = accpool.tile([128, KT, F], BF16, name="x_sb", tag="x_sb")
            x_flat = x_sb[:].rearrange("p a b -> p (a b)")
            A_sb = accpool.tile([128, KT, R], FP8, name="A_sb", tag="A_sb")
            A_flat = A_sb[:].rearrange("p a b -> p (a b)")
            sumsT = accpool.tile([128, R], BF16, name="sumsT")
            wt_sb = cpool.tile([128, FO], BF16, name="wt_sb")
            b_sb = cpool.tile([128, FO], F32, name="b_sb")
            rb_sb = cpool.tile([128, NT], F32, name="rb_sb")

            # ---- all input DMA issued up front, in consumption order,
            # alternating between the two HWDGE queues by byte balance ----
            ba = [0, 1, 3, 7]
            while ba[-1] < KT:
                ba.append(min(KT, ba[-1] + 8))
            bx = [0, 2, 6, 14, 30, 54, KT]
            xfers = []  # (need_slab, kind, k0, k1)
            for k0, k1 in zip(ba, ba[1:]):
                xfers.append((k0, "A", k0, k1))
            for k0, k1 in zip(bx, bx[1:]):
                xfers.append((k0, "x", k0, k1))
            xfers.sort(key=lambda t: (t[0], t[1]))
            qbytes = [0, 0]
            nconst = 0
            for need, kind, k0, k1 in xfers:
                q = 0 if qbytes[0] <= qbytes[1] else 1
                deng = nc.sync if q == 0 else nc.scalar
                if kind == "A":
                    deng.dma_start(A_flat[:, k0 * R : k1 * R], Ad[:, k0 * R : k1 * R])
                    qbytes[q] += (k1 - k0) * R
                else:
                    deng.dma_start(x_flat[:, k0 * F : k1 * F], xtd[:, k0 * F : k1 * F])
                    qbytes[q] += (k1 - k0) * F * 2
                nconst += 1
                if nconst == 4:  # constants early but off the queue heads
                    nc.scalar.dma_start(wt_sb[:], Wd[:])
                    nc.scalar.dma_start(b_sb[:], bd[:])
                    nc.scalar.dma_start(rb_sb[:], rbd[:])
                    qbytes[1] += FO * 2 + FO * 4 * 128 + NT * 4

            # PE warm-up: ~3us of tiny matmuls during the first-chunk DMA
            # wait so the HAM clock gate is at full rate for the real stream.
            warm = cpool.tile([128, 128], BF16, name="warm")
            nc.vector.memset(warm[:], 0.0)
            for _w in range(28):
                nc.tensor.matmul(
                    psall[:16, 3584:3712], warm[:, 0:16], warm[:, 0:128],
                    start=True, stop=True, skip_group_check=True,
                )

            # ---- phase 1 pass A: sumsT[f, d] += x[s, f] * A[s, d] ----
            for k in range(KT):
                st = k == 0
                sp = k == KT - 1
                for (c0, c1) in passA:
                    nc.tensor.matmul(
                        psall[:, c0:c1], x_sb[:, k, :], A_sb[:, k, c0:c1],
                        start=st, stop=sp, skip_group_check=False,
                    )

            # evacuate pass-A tiles (PSUM -> SBUF bf16), alternating engines
            for t in range(8):
                cs = slice(128 * t, 128 * (t + 1))
                if t % 2:
                    nc.scalar.copy(sumsT[:, cs], psall[:, cs])
                else:
                    nc.vector.tensor_scalar_mul(sumsT[:, cs], psall[:, cs], 1.0)

            # ---- phase 1 pass B + phase-3 drain of pass-A tiles ----
            # per node tile t: out_t = (sumsT_t.T @ W) * rb_t + b
            def drain_tile(t):
                m = min(128, S - 128 * t)      # tile 9 is the 98-row remnant
                rows = slice(128 * t, 128 * t + m)
                ps3 = psall[:, 2048 + (t % 3) * 512 : 2048 + (t % 3) * 512 + 512]
                nc.tensor.matmul(ps3[:m, :], sumsT[:, rows], wt_sb[:],
                                 start=True, stop=True, skip_group_check=True)
                ot = p3pool.tile([128, FO], F32, tag="ot")
                if t % 2:
                    nc.vector.scalar_tensor_tensor(
                        ot[:m, :], ps3[:m, :], rb_sb[:m, t : t + 1], b_sb[:m, :],
                        op0=mybir.AluOpType.mult, op1=mybir.AluOpType.add,
                    )
                else:  # GPSIMD cannot read PSUM: ACT evacuates+scales, Pool adds b
                    nc.scalar.mul(ot[:m, :], ps3[:m, :], rb_sb[:m, t : t + 1])
                    nc.gpsimd.tensor_add(ot[:m, :], ot[:m, :], b_sb[:m, :])
                deng = nc.scalar if t % 2 else nc.sync
                deng.dma_start(out[rows, :], ot[:m, :])

            nxt_drain = 0
            for k in range(KT):
                st = k == 0
                sp = k == KT - 1
                for (c0, c1) in passB:
                    nc.tensor.matmul(
                        psall[:, c0:c1], x_sb[:, k, :], A_sb[:, k, c0:c1],
                        start=st, stop=sp, skip_group_check=False,
                    )
                if k % 9 == 8 and nxt_drain < 8:
                    drain_tile(nxt_drain)
                    nxt_drain += 1

            # evacuate + drain the pass-B tiles
            for t in range(8, NT):
                cs = slice(128 * t, min(128 * (t + 1), S))
                if t % 2:
                    nc.scalar.copy(sumsT[:, cs], psall[:, cs])
                else:
                    nc.vector.tensor_scalar_mul(sumsT[:, cs], psall[:, cs], 1.0)
            while nxt_drain < NT:
                drain_tile(nxt_drain)
                nxt_drain += 1

    nc.compile()
    return nc


def _shard_inputs(x32, src, dst, W32, b32, n_cores):
    import ml_dtypes

    N, F = x32.shape
    FO = W32.shape[1]
    S = (N + n_cores - 1) // n_cores
    NT = (S + 127) // 128
    R = NT * 128
    KT = (N + 127) // 128

    # x slabs, feature-minor: xt[p, k, f] = x[128k + p, f], bf16
    xp = np.zeros((KT * 128, F), np.float32)
    xp[:N] = x32
    xt = np.ascontiguousarray(
        xp.reshape(KT, 128, F).transpose(1, 0, 2).astype(ml_dtypes.bfloat16)
    ).reshape(128, KT * F)

    deg = np.bincount(dst, minlength=N)
    rb_full = (1.0 / np.maximum(deg, 1)).astype(np.float32)
    zero_nodes = np.where(deg == 0)[0]

    brep = np.ascontiguousarray(np.tile(b32.reshape(1, -1), (128, 1)))
    Wb = W32.astype(ml_dtypes.bfloat16)

    in_maps = []
    for c in range(n_cores):
        lo = c * S
        hi = min(N, lo + S)
        sel = (dst >= lo) & (dst < hi)
        A = np.zeros((KT * 128, R), np.float32)
        np.add.at(A, (src[sel], dst[sel] - lo), 1.0)
        zn = zero_nodes[(zero_nodes >= lo) & (zero_nodes < hi)]
        if zn.size:  # self-edge: zero-in-degree nodes keep their input
            A[zn, zn - lo] += 1.0
        assert A.max() <= 16, "edge multiplicity too large for fp8e4m3"
        A8 = np.ascontiguousarray(
            A.reshape(KT, 128, R).transpose(1, 0, 2).astype(ml_dtypes.float8_e4m3)
        ).reshape(128, KT * R)
        rb_c = np.ones(R, np.float32)
        rb_c[: hi - lo] = rb_full[lo:hi]
        rb_c = np.ascontiguousarray(rb_c.reshape(NT, 128).T)
        in_maps.append({"xt": xt, "A": A8, "W": Wb, "b": brep, "rb": rb_c})
    return in_maps, R


def _install_ntff_shim():
    """antenv.axon_hooks shim so trace=True can NTFF-profile in this env."""
    import contextlib
    import ctypes
    import sys
    import types

    if "antenv.axon_hooks" in sys.modules:
        return
    so_path = "/opt/axon/libaxon_pjrt.so"
    try:
        lib = ctypes.CDLL(so_path)
        lib.axon_start_nrt_profile.argtypes = [
            ctypes.POINTER(ctypes.c_int64), ctypes.c_size_t]
        lib.axon_start_nrt_profile.restype = ctypes.c_int64
        lib.axon_stop_nrt_profile.argtypes = [ctypes.c_char_p]
        lib.axon_stop_nrt_profile.restype = ctypes.c_int64
    except Exception:
        return

    @contextlib.contextmanager
    def _hook(output_dir, device_ids):
        import jax

        jax.devices()
        if device_ids:
            ids = (ctypes.c_int64 * len(device_ids))(*device_ids)
            rc = lib.axon_start_nrt_profile(ids, len(device_ids))
        else:
            rc = lib.axon_start_nrt_profile(None, 0)
        if rc != 0:
            raise RuntimeError(f"axon_start_nrt_profile rc={rc}")
        try:
            yield
        finally:
            lib.axon_stop_nrt_profile(str(output_dir).encode())

    mod = types.ModuleType("antenv.axon_hooks")
    mod.set_axon_ntff_profile_hook = lambda h: None
    mod.get_axon_ntff_profile_hook = lambda: _hook
    sys.modules["antenv.axon_hooks"] = mod


def kernel(x, src, dst, W, b):
    from concourse import bass_utils

    x32 = np.ascontiguousarray(np.asarray(x), dtype=np.float32)
    W32 = np.ascontiguousarray(np.asarray(W), dtype=np.float32)
    b32 = np.ascontiguousarray(np.asarray(b), dtype=np.float32)
    src = np.asarray(src).astype(np.int64)
    dst = np.asarray(dst).astype(np.int64)
    N, F = x32.shape
    FO = W32.shape[1]
    S = (N + CORES - 1) // CORES

    in_maps, R = _shard_inputs(x32, src, dst, W32, b32, CORES)

    key = (N, F, FO, R)
    if key not in _cache:
        _cache[key] = _build_program(N, F, FO, R, S)
    nc = _cache[key]

    if TRACE:
        _install_ntff_shim()

    last_err = None
    for _attempt in range(2):
        try:
            res = bass_utils.run_bass_kernel_spmd(
                nc, in_maps, core_ids=list(range(CORES)), trace=TRACE
            )
            break
        except Exception as e:  # retry once on transient device errors
            last_err = e
    else:
        raise last_err

    if TRACE and res.exec_time_ns is not None:
        print("HW exec time:", res.exec_time_ns, "ns")

    outs = [np.asarray(r["out"]).reshape(R, FO) for r in res.results]
    full = np.concatenate([o[:S] for o in outs], axis=0)[:N]
    return full.astype(np.float32)


# revision 9
# speedup vs baseline: 1.0034x; 1.0034x over previous
"""GCN layer (gather -> mean-aggregate -> linear) on 8 Trainium2 cores.

Strategy (SPMD, no collectives):
  - Nodes are row-sharded: core c owns dst nodes [c*S, (c+1)*S), S = N/8.
  - Edges are bucketed by dst-owner core on the host and turned into a dense
    per-core adjacency count matrix A[src_node, local_dst] (fp8e4m3 - counts
    are small integers, exact). The per-core segment sum is computed
    FEATURE-MAJOR on the PE array:  sumsT = x.T @ A, with x as bf16 slabs
    (lhsT stationary) and A streaming as the rhs.  This streams R columns
    per src slab (vs 2*F+1 per slab per node-group for the node-major
    orientation) - half the PE cycles of the hi/lo node-major scheme.
  - Degrees (and the zero-in-degree fallback) are folded in on the host:
    rb = 1/max(deg,1) is shipped per node, and zero-degree nodes get a
    self-edge in A so mean==x for them (DGL recv semantics), making the
    device program branch-free.
  - Phase 1 runs in two column passes (cols 0:1024, then 1024:1250).  The
    second pass's matmul stream hides the drain of the first pass's eight
    node tiles: per tile, evacuate PSUM->SBUF (bf16), GEMM
    out = (sumsT_t.T @ W) (sumsT is already the lhsT layout - no PE
    transposes), fused scale(1/deg)+bias on DVE/GPSIMD, and the output DMA.
  - All input DMA is issued up front (A/x are fully SBUF-resident), split
    across the two HWDGE queues in consumption order.
"""

import os

import numpy as np

CORES = 8
TRACE = False           # set by test harness to print HW exec time
_cache = {}


def _build_program(N, F, FO, R, S):
    from concourse import bacc, tile
    from concourse.bass import mybir

    F32 = mybir.dt.float32
    BF16 = mybir.dt.bfloat16
    FP8 = mybir.dt.float8e4
    KT = (N + 127) // 128          # src-node slabs
    NT = (S + 127) // 128          # owned-node tiles per core
    nc = bacc.Bacc(None)

    CL = 1024                      # pass-A columns (8 node tiles)
    CR = S - CL                    # pass-B columns (remnant tiles)
    xtd = nc.dram_tensor("xt", [128, KT * F], BF16, kind="ExternalInput")
    Ald = nc.dram_tensor("Al", [128, KT * CL], FP8, kind="ExternalInput")
    Ard = nc.dram_tensor("Ar", [128, KT * CR], FP8, kind="ExternalInput")
    Wd = nc.dram_tensor("W", [F, FO], BF16, kind="ExternalInput")
    bd = nc.dram_tensor("b", [128, FO], F32, kind="ExternalInput")
    rbd = nc.dram_tensor("rb", [128, NT], F32, kind="ExternalInput")
    out = nc.dram_tensor("out", [R, FO], F32, kind="ExternalOutput")

    # PSUM bank map (each accumulating group owns a 2KB bank, zeroed by
    # its first start=True matmul):
    #   banks 0,1: pass-A sumsT col groups [0:512],[512:1024]
    #   bank  2:   pass-B col group [1024:S]
    #   banks 4,5,6: phase-3 out psum, cycling t%3
    #   bank  7 head: PE warm-up scratch
    psall = nc.alloc_psum_tensor("psall", [128, 4096], F32)

    with tile.TileContext(nc) as tc:
        with (
            tc.tile_pool(name="const", bufs=1) as cpool,
            tc.tile_pool(name="acc", bufs=1) as accpool,
            tc.tile_pool(name="p3", bufs=4) as p3pool,
        ):
            x_sb = accpool.tile([128, KT, F], BF16, name="x_sb", tag="x_sb")
            x_flat = x_sb[:].rearrange("p a b -> p (a b)")
            Al_sb = accpool.tile([128, KT, CL], FP8, name="Al_sb", tag="Al_sb")
            Al_flat = Al_sb[:].rearrange("p a b -> p (a b)")
            Ar_sb = accpool.tile([128, KT, CR], FP8, name="Ar_sb", tag="Ar_sb")
            Ar_flat = Ar_sb[:].rearrange("p a b -> p (a b)")
            sumsT = accpool.tile([128, R], BF16, name="sumsT")
            wt_sb = cpool.tile([128, FO], BF16, name="wt_sb")
            b_sb = cpool.tile([128, FO], F32, name="b_sb")
            rb_sb = cpool.tile([128, NT], F32, name="rb_sb")

            # ---- all input DMA issued up front. sync queue: the pass-A
            # column block of A, streamed in consumption order.  scalar
            # queue: x slabs, constants, then the pass-B block.  The two
            # HWDGE rings drain concurrently (engines round-robin at packet
            # granularity), so pass A is never starved and completes early,
            # letting its 8 node tiles drain under the pass-B stream. ----
            bal = [0, 2] + list(range(10, KT, 8)) + [KT]
            for k0, k1 in zip(bal, bal[1:]):
                nc.sync.dma_start(Al_flat[:, k0 * CL : k1 * CL],
                                  Ald[:, k0 * CL : k1 * CL])
            bx = [0, 2, 10, 26, 50, KT]
            for k0, k1 in zip(bx, bx[1:]):
                nc.scalar.dma_start(x_flat[:, k0 * F : k1 * F],
                                    xtd[:, k0 * F : k1 * F])
            nc.scalar.dma_start(wt_sb[:], Wd[:])
            nc.scalar.dma_start(b_sb[:], bd[:])
            nc.scalar.dma_start(rb_sb[:], rbd[:])
            bar = [0, 26, 52, KT]
            for k0, k1 in zip(bar, bar[1:]):
                nc.scalar.dma_start(Ar_flat[:, k0 * CR : k1 * CR],
                                    Ard[:, k0 * CR : k1 * CR])

            # PE warm-up: ~3us of tiny matmuls during the first-chunk DMA
            # wait so the HAM clock gate is at full rate for the real stream.
            warm = cpool.tile([128, 128], BF16, name="warm")
            nc.vector.memset(warm[:], 0.0)
            for _w in range(24):
                nc.tensor.matmul(
                    psall[:16, 3584:3712], warm[:, 0:16], warm[:, 0:128],
                    start=True, stop=True, skip_group_check=True,
                )

            # ---- phase 1 pass A: sumsT[f, d] += x[s, f] * A[s, d] ----
            for k in range(KT):
                st = k == 0
                sp = k == KT - 1
                for (c0, c1) in [(0, 512), (512, 1024)]:
                    nc.tensor.matmul(
                        psall[:, c0:c1], x_sb[:, k, :], Al_sb[:, k, c0:c1],
                        start=st, stop=sp, skip_group_check=False,
                    )

            # evacuate pass-A tiles (PSUM -> SBUF bf16), alternating engines
            for t in range(8):
                cs = slice(128 * t, 128 * (t + 1))
                if t % 2:
                    nc.scalar.copy(sumsT[:, cs], psall[:, cs])
                else:
                    nc.vector.tensor_scalar_mul(sumsT[:, cs], psall[:, cs], 1.0)

            # ---- phase 1 pass B + phase-3 drain of pass-A tiles ----
            # per node tile t: out_t = (sumsT_t.T @ W) * rb_t + b
            def drain_tile(t):
                m = min(128, S - 128 * t)      # tile 9 is the 98-row remnant
                rows = slice(128 * t, 128 * t + m)
                ps3 = psall[:, 2048 + (t % 3) * 512 : 2048 + (t % 3) * 512 + 512]
                nc.tensor.matmul(ps3[:m, :], sumsT[:, rows], wt_sb[:],
                                 start=True, stop=True, skip_group_check=True)
                ot = p3pool.tile([128, FO], F32, tag="ot")
                if t % 2:
                    nc.vector.scalar_tensor_tensor(
                        ot[:m, :], ps3[:m, :], rb_sb[:m, t : t + 1], b_sb[:m, :],
                        op0=mybir.AluOpType.mult, op1=mybir.AluOpType.add,
                    )
                else:  # GPSIMD cannot read PSUM: ACT evacuates+scales, Pool adds b
                    nc.scalar.mul(ot[:m, :], ps3[:m, :], rb_sb[:m, t : t + 1])
                    nc.gpsimd.tensor_add(ot[:m, :], ot[:m, :], b_sb[:m, :])
                deng = nc.scalar if t % 2 else nc.sync
                deng.dma_start(out[rows, :], ot[:m, :])

            nxt_drain = 0
            for k in range(KT):
                st = k == 0
                sp = k == KT - 1
                nc.tensor.matmul(
                    psall[:, 1024 : 1024 + CR], x_sb[:, k, :], Ar_sb[:, k, :],
                    start=st, stop=sp, skip_group_check=False,
                )
                if k % 9 == 8 and nxt_drain < 8:
                    drain_tile(nxt_drain)
                    nxt_drain += 1

            # evacuate + drain the pass-B tiles
            for t in range(8, NT):
                cs = slice(128 * t, min(128 * (t + 1), S))
                if t % 2:
                    nc.scalar.copy(sumsT[:, cs], psall[:, cs])
                else:
                    nc.vector.tensor_scalar_mul(sumsT[:, cs], psall[:, cs], 1.0)
            while nxt_drain < NT:
                drain_tile(nxt_drain)
                nxt_drain += 1

    nc.compile()
    return nc


def _shard_inputs(x32, src, dst, W32, b32, n_cores):
    import ml_dtypes

    N, F = x32.shape
    FO = W32.shape[1]
    S = (N + n_cores - 1) // n_cores
    NT = (S + 127) // 128
    R = NT * 128
    KT = (N + 127) // 128

    # x slabs, feature-minor: xt[p, k, f] = x[128k + p, f], bf16
    xp = np.zeros((KT * 128, F), np.float32)
    xp[:N] = x32
    xt = np.ascontiguousarray(
        xp.reshape(KT, 128, F).transpose(1, 0, 2).astype(ml_dtypes.bfloat16)
    ).reshape(128, KT * F)

    deg = np.bincount(dst, minlength=N)
    rb_full = (1.0 / np.maximum(deg, 1)).astype(np.float32)
    zero_nodes = np.where(deg == 0)[0]

    brep = np.ascontiguousarray(np.tile(b32.reshape(1, -1), (128, 1)))
    Wb = W32.astype(ml_dtypes.bfloat16)

    in_maps = []
    for c in range(n_cores):
        lo = c * S
        hi = min(N, lo + S)
        sel = (dst >= lo) & (dst < hi)
        A = np.zeros((KT * 128, R), np.float32)
        np.add.at(A, (src[sel], dst[sel] - lo), 1.0)
        zn = zero_nodes[(zero_nodes >= lo) & (zero_nodes < hi)]
        if zn.size:  # self-edge: zero-in-degree nodes keep their input
            A[zn, zn - lo] += 1.0
        assert A.max() <= 16, "edge multiplicity too large for fp8e4m3"
        CL = 1024
        A3 = A.reshape(KT, 128, R).transpose(1, 0, 2).astype(ml_dtypes.float8_e4m3)
        Al = np.ascontiguousarray(A3[:, :, :CL]).reshape(128, KT * CL)
        Ar = np.ascontiguousarray(A3[:, :, CL:S]).reshape(128, KT * (S - CL))
        rb_c = np.ones(R, np.float32)
        rb_c[: hi - lo] = rb_full[lo:hi]
        rb_c = np.ascontiguousarray(rb_c.reshape(NT, 128).T)
        in_maps.append({"xt": xt, "Al": Al, "Ar": Ar, "W": Wb, "b": brep,
                        "rb": rb_c})
    return in_maps, R


def _install_ntff_shim():
    """antenv.axon_hooks shim so trace=True can NTFF-profile in this env."""
    import contextlib
    import ctypes
    import sys
    import types

    if "antenv.axon_hooks" in sys.modules:
        return
    so_path = "/opt/axon/libaxon_pjrt.so"
    try:
        lib = ctypes.CDLL(so_path)
        lib.axon_start_nrt_profile.argtypes = [
            ctypes.POINTER(ctypes.c_int64), ctypes.c_size_t]
        lib.axon_start_nrt_profile.restype = ctypes.c_int64
        lib.axon_stop_nrt_profile.argtypes = [ctypes.c_char_p]
        lib.axon_stop_nrt_profile.restype = ctypes.c_int64
    except Exception:
        return

    @contextlib.contextmanager
    def _hook(output_dir, device_ids):
        import jax

        jax.devices()
        if device_ids:
            ids = (ctypes.c_int64 * len(device_ids))(*device_ids)
            rc = lib.axon_start_nrt_profile(ids, len(device_ids))
        else:
            rc = lib.axon_start_nrt_profile(None, 0)
        if rc != 0:
            raise RuntimeError(f"axon_start_nrt_profile rc={rc}")
        try:
            yield
        finally:
            lib.axon_stop_nrt_profile(str(output_dir).encode())

    mod = types.ModuleType("antenv.axon_hooks")
    mod.set_axon_ntff_profile_hook = lambda h: None
    mod.get_axon_ntff_profile_hook = lambda: _hook
    sys.modules["antenv.axon_hooks"] = mod


def kernel(x, src, dst, W, b):
    from concourse import bass_utils

    x32 = np.ascontiguousarray(np.asarray(x), dtype=np.float32)
    W32 = np.ascontiguousarray(np.asarray(W), dtype=np.float32)
    b32 = np.ascontiguousarray(np.asarray(b), dtype=np.float32)
    src = np.asarray(src).astype(np.int64)
    dst = np.asarray(dst).astype(np.int64)
    N, F = x32.shape
    FO = W32.shape[1]
    S = (N + CORES - 1) // CORES

    in_maps, R = _shard_inputs(x32, src, dst, W32, b32, CORES)

    key = (N, F, FO, R)
    if key not in _cache:
        _cache[key] = _build_program(N, F, FO, R, S)
    nc = _cache[key]

    if TRACE:
        _install_ntff_shim()

    last_err = None
    for _attempt in range(2):
        try:
            res = bass_utils.run_bass_kernel_spmd(
                nc, in_maps, core_ids=list(range(CORES)), trace=TRACE
            )
            break
        except Exception as e:  # retry once on transient device errors
            last_err = e
    else:
        raise last_err

    if TRACE and res.exec_time_ns is not None:
        print("HW exec time:", res.exec_time_ns, "ns")

    outs = [np.asarray(r["out"]).reshape(R, FO) for r in res.results]
    full = np.concatenate([o[:S] for o in outs], axis=0)[:N]
    return full.astype(np.float32)


# revision 10
# speedup vs baseline: 1.0086x; 1.0052x over previous
"""GCN layer (gather -> mean-aggregate -> linear) on 8 Trainium2 cores.

Strategy (SPMD, no collectives):
  - Nodes are row-sharded: core c owns dst nodes [c*S, (c+1)*S), S = N/8.
  - Edges are bucketed by dst-owner core on the host and turned into a dense
    per-core adjacency count matrix A[src_node, local_dst] (fp8e4m3 - counts
    are small integers, exact). The per-core segment sum is computed
    FEATURE-MAJOR on the PE array:  sumsT = x.T @ A, with x as bf16 slabs
    (lhsT stationary) and A streaming as the rhs.  This streams S columns
    per src slab (vs 2*F+1 per slab per node-group for the node-major
    orientation) - half the PE cycles of the hi/lo node-major scheme.
  - Degrees (and the zero-in-degree fallback) are folded in on the host:
    rb = 1/max(deg,1) is shipped per node, and zero-degree nodes get a
    self-edge in A so mean==x for them (DGL recv semantics), making the
    device program branch-free.
  - Inputs stream on ONE HWDGE queue in exact consumption order (x slab
    chunk, then A slab chunk, interleaved) so the PE is never starved;
    constants ride the other queue.
  - Drain per node tile: evacuate PSUM->SBUF (bf16), GEMM
    out_t = (sumsT_t.T @ W) (sumsT is already the lhsT layout - no PE
    transposes), fused scale(1/deg)+bias split across DVE / ACT+Pool,
    output DMA alternating queues.
"""

import os

import numpy as np

CORES = 8
TRACE = False           # set by test harness to print HW exec time
_cache = {}


def _build_program(N, F, FO, R, S):
    from concourse import bacc, tile
    from concourse.bass import mybir

    F32 = mybir.dt.float32
    BF16 = mybir.dt.bfloat16
    FP8 = mybir.dt.float8e4
    KT = (N + 127) // 128          # src-node slabs
    NT = (S + 127) // 128          # owned-node tiles per core
    nc = bacc.Bacc(None)

    xtd = nc.dram_tensor("xt", [128, KT * F], BF16, kind="ExternalInput")
    Ad = nc.dram_tensor("A", [128, KT * S], FP8, kind="ExternalInput")
    Wd = nc.dram_tensor("W", [F, FO], BF16, kind="ExternalInput")
    bd = nc.dram_tensor("b", [128, FO], F32, kind="ExternalInput")
    rbd = nc.dram_tensor("rb", [128, NT], F32, kind="ExternalInput")
    out = nc.dram_tensor("out", [R, FO], F32, kind="ExternalOutput")

    # PSUM bank map (each accumulating group owns a 2KB bank, zeroed by
    # its first start=True matmul):
    #   banks 0..2: phase-1 sumsT col groups [0:512],[512:1024],[1024:S]
    #   banks 3..6: phase-3 out psum, cycling t%4
    #   bank  7 head: PE warm-up scratch
    psall = nc.alloc_psum_tensor("psall", [128, 4096], F32)
    cgrps = [(c0, min(S, c0 + 512)) for c0 in range(0, S, 512)]

    with tile.TileContext(nc) as tc:
        with (
            tc.tile_pool(name="const", bufs=1) as cpool,
            tc.tile_pool(name="acc", bufs=1) as accpool,
            tc.tile_pool(name="p3", bufs=5) as p3pool,
        ):
            x_sb = accpool.tile([128, KT, F], BF16, name="x_sb", tag="x_sb")
            x_flat = x_sb[:].rearrange("p a b -> p (a b)")
            A_sb = accpool.tile([128, KT, S], FP8, name="A_sb", tag="A_sb")
            A_flat = A_sb[:].rearrange("p a b -> p (a b)")
            sumsT = accpool.tile([128, S], BF16, name="sumsT")
            wt_sb = cpool.tile([128, FO], BF16, name="wt_sb")
            b_sb = cpool.tile([128, FO], F32, name="b_sb")
            rb_sb = cpool.tile([128, NT], F32, name="rb_sb")

            # ---- all input DMA up front on the sync queue, in exact
            # consumption order (x slab chunk then A slab chunk) ----
            bounds = [0, 1, 2, 4, 8, 14, 22, 30, 38, 46, 54, 62, 70, KT]
            for k0, k1 in zip(bounds, bounds[1:]):
                nc.sync.dma_start(x_flat[:, k0 * F : k1 * F],
                                  xtd[:, k0 * F : k1 * F])
                nc.sync.dma_start(A_flat[:, k0 * S : k1 * S],
                                  Ad[:, k0 * S : k1 * S])
            nc.scalar.dma_start(wt_sb[:], Wd[:])
            nc.scalar.dma_start(b_sb[:], bd[:])
            nc.scalar.dma_start(rb_sb[:], rbd[:])

            # PE warm-up: tiny matmuls during the first-chunk DMA wait so
            # the HAM clock gate is at full rate for the real stream.
            warm = cpool.tile([128, 128], BF16, name="warm")
            nc.vector.memset(warm[:], 0.0)
            for _w in range(20):
                nc.tensor.matmul(
                    psall[:16, 3584:3712], warm[:, 0:16], warm[:, 0:128],
                    start=True, stop=True, skip_group_check=True,
                )

            # ---- phase 1: sumsT[f, d] += x[s, f] * A[s, d], slab-major ----
            for k in range(KT):
                st = k == 0
                sp = k == KT - 1
                for (c0, c1) in cgrps:
                    nc.tensor.matmul(
                        psall[:, c0:c1], x_sb[:, k, :], A_sb[:, k, c0:c1],
                        start=st, stop=sp, skip_group_check=False,
                    )

            # ---- drain per node tile: evac, GEMM, scale+bias, store ----
            for t in range(NT):
                m = min(128, S - 128 * t)      # last tile is a remnant
                rows = slice(128 * t, 128 * t + m)
                if t % 2:
                    nc.scalar.copy(sumsT[:, rows], psall[:, rows])
                else:
                    nc.vector.tensor_scalar_mul(sumsT[:, rows],
                                                psall[:, rows], 1.0)
                ps3 = psall[:, 1536 + (t % 4) * 512 : 2048 + (t % 4) * 512]
                nc.tensor.matmul(ps3[:m, :], sumsT[:, rows], wt_sb[:],
                                 start=True, stop=True, skip_group_check=True)
                ot = p3pool.tile([128, FO], F32, tag="ot")
                if t % 2:
                    nc.vector.scalar_tensor_tensor(
                        ot[:m, :], ps3[:m, :], rb_sb[:m, t : t + 1], b_sb[:m, :],
                        op0=mybir.AluOpType.mult, op1=mybir.AluOpType.add,
                    )
                else:  # GPSIMD cannot read PSUM: ACT evacuates+scales, Pool adds b
                    nc.scalar.mul(ot[:m, :], ps3[:m, :], rb_sb[:m, t : t + 1])
                    nc.gpsimd.tensor_add(ot[:m, :], ot[:m, :], b_sb[:m, :])
                deng = nc.scalar if t % 2 else nc.sync
                deng.dma_start(out[rows, :], ot[:m, :])

    nc.compile()
    return nc


def _shard_inputs(x32, src, dst, W32, b32, n_cores):
    import ml_dtypes

    N, F = x32.shape
    FO = W32.shape[1]
    S = (N + n_cores - 1) // n_cores
    NT = (S + 127) // 128
    R = NT * 128
    KT = (N + 127) // 128

    # x slabs, feature-minor: xt[p, k, f] = x[128k + p, f], bf16
    xp = np.zeros((KT * 128, F), np.float32)
    xp[:N] = x32
    xt = np.ascontiguousarray(
        xp.reshape(KT, 128, F).transpose(1, 0, 2).astype(ml_dtypes.bfloat16)
    ).reshape(128, KT * F)

    deg = np.bincount(dst, minlength=N)
    rb_full = (1.0 / np.maximum(deg, 1)).astype(np.float32)
    zero_nodes = np.where(deg == 0)[0]

    brep = np.ascontiguousarray(np.tile(b32.reshape(1, -1), (128, 1)))
    Wb = W32.astype(ml_dtypes.bfloat16)

    in_maps = []
    for c in range(n_cores):
        lo = c * S
        hi = min(N, lo + S)
        sel = (dst >= lo) & (dst < hi)
        A = np.zeros((KT * 128, S), np.float32)
        np.add.at(A, (src[sel], dst[sel] - lo), 1.0)
        zn = zero_nodes[(zero_nodes >= lo) & (zero_nodes < hi)]
        if zn.size:  # self-edge: zero-in-degree nodes keep their input
            A[zn, zn - lo] += 1.0
        assert A.max() <= 16, "edge multiplicity too large for fp8e4m3"
        A8 = np.ascontiguousarray(
            A.reshape(KT, 128, S).transpose(1, 0, 2).astype(ml_dtypes.float8_e4m3)
        ).reshape(128, KT * S)
        rb_c = np.ones(R, np.float32)
        rb_c[: hi - lo] = rb_full[lo:hi]
        rb_c = np.ascontiguousarray(rb_c.reshape(NT, 128).T)
        in_maps.append({"xt": xt, "A": A8, "W": Wb, "b": brep, "rb": rb_c})
    return in_maps, R


def _install_ntff_shim():
    """antenv.axon_hooks shim so trace=True can NTFF-profile in this env."""
    import contextlib
    import ctypes
    import sys
    import types

    if "antenv.axon_hooks" in sys.modules:
        return
    so_path = "/opt/axon/libaxon_pjrt.so"
    try:
        lib = ctypes.CDLL(so_path)
        lib.axon_start_nrt_profile.argtypes = [
            ctypes.POINTER(ctypes.c_int64), ctypes.c_size_t]
        lib.axon_start_nrt_profile.restype = ctypes.c_int64
        lib.axon_stop_nrt_profile.argtypes = [ctypes.c_char_p]
        lib.axon_stop_nrt_profile.restype = ctypes.c_int64
    except Exception:
        return

    @contextlib.contextmanager
    def _hook(output_dir, device_ids):
        import jax

        jax.devices()
        if device_ids:
            ids = (ctypes.c_int64 * len(device_ids))(*device_ids)
            rc = lib.axon_start_nrt_profile(ids, len(device_ids))
        else:
            rc = lib.axon_start_nrt_profile(None, 0)
        if rc != 0:
            raise RuntimeError(f"axon_start_nrt_profile rc={rc}")
        try:
            yield
        finally:
            lib.axon_stop_nrt_profile(str(output_dir).encode())

    mod = types.ModuleType("antenv.axon_hooks")
    mod.set_axon_ntff_profile_hook = lambda h: None
    mod.get_axon_ntff_profile_hook = lambda: _hook
    sys.modules["antenv.axon_hooks"] = mod


def kernel(x, src, dst, W, b):
    from concourse import bass_utils

    x32 = np.ascontiguousarray(np.asarray(x), dtype=np.float32)
    W32 = np.ascontiguousarray(np.asarray(W), dtype=np.float32)
    b32 = np.ascontiguousarray(np.asarray(b), dtype=np.float32)
    src = np.asarray(src).astype(np.int64)
    dst = np.asarray(dst).astype(np.int64)
    N, F = x32.shape
    FO = W32.shape[1]
    S = (N + CORES - 1) // CORES

    in_maps, R = _shard_inputs(x32, src, dst, W32, b32, CORES)

    key = (N, F, FO, R)
    if key not in _cache:
        _cache[key] = _build_program(N, F, FO, R, S)
    nc = _cache[key]

    if TRACE:
        _install_ntff_shim()

    last_err = None
    for _attempt in range(2):
        try:
            res = bass_utils.run_bass_kernel_spmd(
                nc, in_maps, core_ids=list(range(CORES)), trace=TRACE
            )
            break
        except Exception as e:  # retry once on transient device errors
            last_err = e
    else:
        raise last_err

    if TRACE and res.exec_time_ns is not None:
        print("HW exec time:", res.exec_time_ns, "ns")

    outs = [np.asarray(r["out"]).reshape(R, FO) for r in res.results]
    full = np.concatenate([o[:S] for o in outs], axis=0)[:N]
    return full.astype(np.float32)


# revision 13
# speedup vs baseline: 1.0387x; 1.0298x over previous
"""GCN layer (gather -> mean-aggregate -> linear) on 8 Trainium2 cores.

Strategy (SPMD, no collectives):
  - Nodes are row-sharded: core c owns dst nodes [c*S, (c+1)*S), S = N/8.
  - Edges are bucketed by dst-owner core on the host and turned into a dense
    per-core adjacency count matrix A[src_node, local_dst] (fp8e4m3 - counts
    are small integers, exact). The per-core segment sum is computed
    FEATURE-MAJOR on the PE array:  sumsT = x.T @ A, with x as bf16 slabs
    (lhsT stationary) and A streaming as the rhs.  This streams S columns
    per src slab (vs 2*F+1 per slab per node-group for the node-major
    orientation) - half the PE cycles of the hi/lo node-major scheme.
  - Degrees (and the zero-in-degree fallback) are folded in on the host:
    rb = 1/max(deg,1) is shipped per node, and zero-degree nodes get a
    self-edge in A so mean==x for them (DGL recv semantics), making the
    device program branch-free.
  - Inputs stream on ONE HWDGE queue in exact consumption order (x slab
    chunk, then A slab chunk, interleaved) so the PE is never starved;
    constants ride the other queue.
  - Drain per node tile: evacuate PSUM->SBUF (bf16), GEMM
    out_t = (sumsT_t.T @ W) (sumsT is already the lhsT layout - no PE
    transposes), fused scale(1/deg)+bias split across DVE / ACT+Pool,
    output DMA alternating queues.
"""

import os

import numpy as np

CORES = 8
TRACE = False           # set by test harness to print HW exec time
_cache = {}


def _build_program(N, F, FO, R, S):
    from concourse import bacc, tile
    from concourse.bass import mybir

    F32 = mybir.dt.float32
    BF16 = mybir.dt.bfloat16
    FP8 = mybir.dt.float8e4
    KT = (N + 127) // 128          # src-node slabs
    NT = (S + 127) // 128          # owned-node tiles per core
    nc = bacc.Bacc(None)

    xtd = nc.dram_tensor("xt", [128, KT * F], BF16, kind="ExternalInput")
    Ad = nc.dram_tensor("A", [128, KT * S], FP8, kind="ExternalInput")
    Wd = nc.dram_tensor("W", [F, FO], BF16, kind="ExternalInput")
    bd = nc.dram_tensor("b", [128, FO], F32, kind="ExternalInput")
    rbd = nc.dram_tensor("rb", [128, NT], F32, kind="ExternalInput")
    out = nc.dram_tensor("out", [R, FO], F32, kind="ExternalOutput")

    # PSUM bank map (each accumulating group owns a 2KB bank, zeroed by
    # its first start=True matmul):
    #   banks 0..2: phase-1 sumsT col groups [0:512],[512:1024],[1024:S]
    #   banks 3..6: phase-3 out psum, cycling t%4
    #   bank  7 head: PE warm-up scratch
    psall = nc.alloc_psum_tensor("psall", [128, 4096], F32)
    cgrps = [(c0, min(S, c0 + 512)) for c0 in range(0, S, 512)]

    with tile.TileContext(nc) as tc:
        with (
            tc.tile_pool(name="const", bufs=1) as cpool,
            tc.tile_pool(name="acc", bufs=1) as accpool,
            tc.tile_pool(name="p3", bufs=10) as p3pool,
        ):
            x_sb = accpool.tile([128, KT, F], BF16, name="x_sb", tag="x_sb")
            x_flat = x_sb[:].rearrange("p a b -> p (a b)")
            A_sb = accpool.tile([128, KT, S], FP8, name="A_sb", tag="A_sb")
            A_flat = A_sb[:].rearrange("p a b -> p (a b)")
            sumsT = accpool.tile([128, S], BF16, name="sumsT")
            wt_sb = cpool.tile([128, FO], BF16, name="wt_sb")
            b_sb = cpool.tile([128, FO], F32, name="b_sb")
            rb_sb = cpool.tile([128, NT], F32, name="rb_sb")

            # ---- all input DMA up front on the sync queue, in exact
            # consumption order (x slab chunk then A slab chunk) ----
            bounds = [0, 4, 12, 24, 40, 60, KT]
            for k0, k1 in zip(bounds, bounds[1:]):
                nc.sync.dma_start(x_flat[:, k0 * F : k1 * F],
                                  xtd[:, k0 * F : k1 * F])
                nc.sync.dma_start(A_flat[:, k0 * S : k1 * S],
                                  Ad[:, k0 * S : k1 * S])
            nc.scalar.dma_start(wt_sb[:], Wd[:])
            nc.scalar.dma_start(b_sb[:], bd[:])
            nc.scalar.dma_start(rb_sb[:], rbd[:])

            # PE warm-up: tiny matmuls during the first-chunk DMA wait so
            # the HAM clock gate is at full rate for the real stream.
            warm = cpool.tile([128, 128], BF16, name="warm")
            nc.vector.memset(warm[:], 0.0)
            for _w in range(20):
                nc.tensor.matmul(
                    psall[:16, 3584:3712], warm[:, 0:16], warm[:, 0:128],
                    start=True, stop=True, skip_group_check=True,
                )

            # ---- phase 1: sumsT[f, d] += x[s, f] * A[s, d], slab-major ----
            for k in range(KT):
                st = k == 0
                sp = k == KT - 1
                for (c0, c1) in cgrps:
                    nc.tensor.matmul(
                        psall[:, c0:c1], x_sb[:, k, :], A_sb[:, k, c0:c1],
                        start=st, stop=sp, skip_group_check=False,
                    )

            # ---- drain per node tile: evac, GEMM, scale+bias, store.
            # Emission order (= scheduler priority) keeps every engine's
            # queue dependency-monotone: all evacs, then the GEMM/scale
            # chains, then all output DMAs - no head-of-line blocking. ----
            for t in range(NT):
                m = min(128, S - 128 * t)      # last tile is a remnant
                rows = slice(128 * t, 128 * t + m)
                if t % 2:
                    nc.scalar.copy(sumsT[:, rows], psall[:, rows])
                else:
                    nc.vector.tensor_scalar_mul(sumsT[:, rows],
                                                psall[:, rows], 1.0)
            ots = []
            for t in range(NT):
                m = min(128, S - 128 * t)
                rows = slice(128 * t, 128 * t + m)
                ps3 = psall[:, 1536 + (t % 4) * 512 : 2048 + (t % 4) * 512]
                nc.tensor.matmul(ps3[:m, :], sumsT[:, rows], wt_sb[:],
                                 start=True, stop=True, skip_group_check=True)
                ot = p3pool.tile([128, FO], F32, tag="ot")
                if t % 2:
                    nc.vector.scalar_tensor_tensor(
                        ot[:m, :], ps3[:m, :], rb_sb[:m, t : t + 1], b_sb[:m, :],
                        op0=mybir.AluOpType.mult, op1=mybir.AluOpType.add,
                    )
                else:  # GPSIMD cannot read PSUM: ACT evacuates+scales, Pool adds b
                    nc.scalar.mul(ot[:m, :], ps3[:m, :], rb_sb[:m, t : t + 1])
                    nc.gpsimd.tensor_add(ot[:m, :], ot[:m, :], b_sb[:m, :])
                ots.append((ot, rows, m))
            for t, (ot, rows, m) in enumerate(ots):
                deng = nc.scalar if t % 2 else nc.sync
                deng.dma_start(out[rows, :], ot[:m, :])

    nc.compile()
    return nc


def _shard_inputs(x32, src, dst, W32, b32, n_cores):
    import ml_dtypes

    N, F = x32.shape
    FO = W32.shape[1]
    S = (N + n_cores - 1) // n_cores
    NT = (S + 127) // 128
    R = NT * 128
    KT = (N + 127) // 128

    # x slabs, feature-minor: xt[p, k, f] = x[128k + p, f], bf16
    xp = np.zeros((KT * 128, F), np.float32)
    xp[:N] = x32
    xt = np.ascontiguousarray(
        xp.reshape(KT, 128, F).transpose(1, 0, 2).astype(ml_dtypes.bfloat16)
    ).reshape(128, KT * F)

    deg = np.bincount(dst, minlength=N)
    rb_full = (1.0 / np.maximum(deg, 1)).astype(np.float32)
    zero_nodes = np.where(deg == 0)[0]

    brep = np.ascontiguousarray(np.tile(b32.reshape(1, -1), (128, 1)))
    Wb = W32.astype(ml_dtypes.bfloat16)

    in_maps = []
    for c in range(n_cores):
        lo = c * S
        hi = min(N, lo + S)
        sel = (dst >= lo) & (dst < hi)
        A = np.zeros((KT * 128, S), np.float32)
        np.add.at(A, (src[sel], dst[sel] - lo), 1.0)
        zn = zero_nodes[(zero_nodes >= lo) & (zero_nodes < hi)]
        if zn.size:  # self-edge: zero-in-degree nodes keep their input
            A[zn, zn - lo] += 1.0
        assert A.max() <= 16, "edge multiplicity too large for fp8e4m3"
        A8 = np.ascontiguousarray(
            A.reshape(KT, 128, S).transpose(1, 0, 2).astype(ml_dtypes.float8_e4m3)
        ).reshape(128, KT * S)
        rb_c = np.ones(R, np.float32)
        rb_c[: hi - lo] = rb_full[lo:hi]
        rb_c = np.ascontiguousarray(rb_c.reshape(NT, 128).T)
        in_maps.append({"xt": xt, "A": A8, "W": Wb, "b": brep, "rb": rb_c})
    return in_maps, R


def _install_ntff_shim():
    """antenv.axon_hooks shim so trace=True can NTFF-profile in this env."""
    import contextlib
    import ctypes
    import sys
    import types

    if "antenv.axon_hooks" in sys.modules:
        return
    so_path = "/opt/axon/libaxon_pjrt.so"
    try:
        lib = ctypes.CDLL(so_path)
        lib.axon_start_nrt_profile.argtypes = [
            ctypes.POINTER(ctypes.c_int64), ctypes.c_size_t]
        lib.axon_start_nrt_profile.restype = ctypes.c_int64
        lib.axon_stop_nrt_profile.argtypes = [ctypes.c_char_p]
        lib.axon_stop_nrt_profile.restype = ctypes.c_int64
    except Exception:
        return

    @contextlib.contextmanager
    def _hook(output_dir, device_ids):
        import jax

        jax.devices()
        if device_ids:
            ids = (ctypes.c_int64 * len(device_ids))(*device_ids)
            rc = lib.axon_start_nrt_profile(ids, len(device_ids))
        else:
            rc = lib.axon_start_nrt_profile(None, 0)
        if rc != 0:
            raise RuntimeError(f"axon_start_nrt_profile rc={rc}")
        try:
            yield
        finally:
            lib.axon_stop_nrt_profile(str(output_dir).encode())

    mod = types.ModuleType("antenv.axon_hooks")
    mod.set_axon_ntff_profile_hook = lambda h: None
    mod.get_axon_ntff_profile_hook = lambda: _hook
    sys.modules["antenv.axon_hooks"] = mod


def kernel(x, src, dst, W, b):
    from concourse import bass_utils

    x32 = np.ascontiguousarray(np.asarray(x), dtype=np.float32)
    W32 = np.ascontiguousarray(np.asarray(W), dtype=np.float32)
    b32 = np.ascontiguousarray(np.asarray(b), dtype=np.float32)
    src = np.asarray(src).astype(np.int64)
    dst = np.asarray(dst).astype(np.int64)
    N, F = x32.shape
    FO = W32.shape[1]
    S = (N + CORES - 1) // CORES

    in_maps, R = _shard_inputs(x32, src, dst, W32, b32, CORES)

    key = (N, F, FO, R)
    if key not in _cache:
        _cache[key] = _build_program(N, F, FO, R, S)
    nc = _cache[key]

    if TRACE:
        _install_ntff_shim()

    last_err = None
    for _attempt in range(2):
        try:
            res = bass_utils.run_bass_kernel_spmd(
                nc, in_maps, core_ids=list(range(CORES)), trace=TRACE
            )
            break
        except Exception as e:  # retry once on transient device errors
            last_err = e
    else:
        raise last_err

    if TRACE and res.exec_time_ns is not None:
        print("HW exec time:", res.exec_time_ns, "ns")

    outs = [np.asarray(r["out"]).reshape(R, FO) for r in res.results]
    full = np.concatenate([o[:S] for o in outs], axis=0)[:N]
    return full.astype(np.float32)


# revision 25
# speedup vs baseline: 1.0663x; 1.0266x over previous
"""GCN layer (gather -> mean-aggregate -> linear) on 8 Trainium2 cores.

Strategy (SPMD, no collectives):
  - Nodes are row-sharded: core c owns dst nodes [c*S, (c+1)*S), S = N/8.
  - Edges are bucketed by dst-owner core on the host and turned into a dense
    per-core adjacency count matrix A[src_node, local_dst] (fp8e4m3 - counts
    are small integers, exact). The per-core segment sum is computed
    FEATURE-MAJOR on the PE array:  sumsT = x.T @ A, with x as bf16 slabs
    (lhsT stationary) and A streaming as the rhs.  This streams S columns
    per src slab (vs 2*F+1 per slab per node-group for the node-major
    orientation) - half the PE cycles of the hi/lo node-major scheme.
  - Degrees (and the zero-in-degree fallback) are folded in on the host:
    rb = 1/max(deg,1) is shipped per node, and zero-degree nodes get a
    self-edge in A so mean==x for them (DGL recv semantics), making the
    device program branch-free.
  - Inputs stream on ONE HWDGE queue in exact consumption order (x slab
    chunk, then A slab chunk, interleaved) so the PE is never starved;
    constants ride the other queue.
  - Drain per node tile: evacuate PSUM->SBUF (bf16), GEMM
    out_t = (sumsT_t.T @ W) (sumsT is already the lhsT layout - no PE
    transposes), fused scale(1/deg)+bias split across DVE / ACT+Pool,
    output DMA alternating queues.
"""

import os

import numpy as np

CORES = 8
TRACE = False           # set by test harness to print HW exec time
_cache = {}


def _build_program(N, F, FO, R, S):
    from concourse import bacc, tile
    from concourse.bass import mybir

    F32 = mybir.dt.float32
    BF16 = mybir.dt.bfloat16
    FP8 = mybir.dt.float8e4
    KT = (N + 127) // 128          # src-node slabs
    NT = (S + 127) // 128          # owned-node tiles per core
    nc = bacc.Bacc(None)

    xtd = nc.dram_tensor("xt", [128, KT * F], BF16, kind="ExternalInput")
    Ad = nc.dram_tensor("A", [128, KT * S], FP8, kind="ExternalInput")
    Wd = nc.dram_tensor("W", [F, FO], BF16, kind="ExternalInput")
    bd = nc.dram_tensor("b", [128, FO], BF16, kind="ExternalInput")
    rbd = nc.dram_tensor("rb", [128, NT], F32, kind="ExternalInput")
    dbd = nc.dram_tensor("db", [128, S], BF16, kind="ExternalInput")
    out = nc.dram_tensor("out", [R, FO], F32, kind="ExternalOutput")

    # PSUM bank map (each accumulating group owns a 2KB bank, zeroed by
    # its first start=True matmul):
    #   banks 0..2: phase-1 sumsT col groups [0:512],[512:1024],[1024:S]
    #   banks 3..6: phase-3 out psum, cycling t%4
    #   bank  7 head: PE warm-up scratch
    psall = nc.alloc_psum_tensor("psall", [128, 4096], F32)
    cgrps = [(c0, min(S, c0 + 512)) for c0 in range(0, S, 512)]

    with tile.TileContext(nc) as tc:
        with (
            tc.tile_pool(name="const", bufs=1) as cpool,
            tc.tile_pool(name="acc", bufs=1) as accpool,
            tc.tile_pool(name="p3", bufs=10) as p3pool,
        ):
            x_sb = accpool.tile([128, KT, F], BF16, name="x_sb", tag="x_sb")
            x_flat = x_sb[:].rearrange("p a b -> p (a b)")
            A_sb = accpool.tile([128, KT, S], FP8, name="A_sb", tag="A_sb")
            A_flat = A_sb[:].rearrange("p a b -> p (a b)")
            sumsT = accpool.tile([128, S], BF16, name="sumsT")
            wt_sb = cpool.tile([128, FO], BF16, name="wt_sb")
            b_sb = cpool.tile([128, FO], BF16, name="b_sb")
            rb_sb = cpool.tile([128, NT], F32, name="rb_sb")
            db_sb = cpool.tile([128, S], BF16, name="db_sb")

            # ---- all input DMA up front on the sync queue, in exact
            # consumption order (x slab chunk then A slab chunk) ----
            bounds = [0, 1, 3, 7, 15, 27, 43, 61, KT]
            for k0, k1 in zip(bounds, bounds[1:]):
                nc.sync.dma_start(x_flat[:, k0 * F : k1 * F],
                                  xtd[:, k0 * F : k1 * F])
                nc.sync.dma_start(A_flat[:, k0 * S : k1 * S],
                                  Ad[:, k0 * S : k1 * S])
            nc.scalar.dma_start(wt_sb[:], Wd[:])
            nc.scalar.dma_start(b_sb[:], bd[:])
            nc.scalar.dma_start(rb_sb[:], rbd[:])
            nc.scalar.dma_start(db_sb[:], dbd[:])

            # PE warm-up: tiny matmuls during the first-chunk DMA wait so
            # the HAM clock gate is at full rate for the real stream.
            warm = cpool.tile([128, 128], BF16, name="warm")
            nc.vector.memset(warm[:], 0.0)
            for _w in range(20):
                nc.tensor.matmul(
                    psall[:16, 3584:3712], warm[:, 0:16], warm[:, 0:128],
                    start=True, stop=True, skip_group_check=True,
                )

            # ---- phase 1: sumsT[f, d] += x[s, f] * A[s, d], slab-major ----
            for k in range(KT):
                st = k == 0
                sp = k == KT - 1
                for (c0, c1) in cgrps:
                    nc.tensor.matmul(
                        psall[:, c0:c1], x_sb[:, k, :], A_sb[:, k, c0:c1],
                        start=st, stop=sp, skip_group_check=False,
                    )

            # ---- drain per node tile: evac, GEMM, scale+bias, store.
            # Emission order (= scheduler priority) keeps every engine's
            # queue dependency-monotone: all evacs, then the GEMM/scale
            # chains, then all output DMAs - no head-of-line blocking. ----
            for t in range(NT):
                m = min(128, S - 128 * t)      # last tile is a remnant
                rows = slice(128 * t, 128 * t + m)
                if t % 2:
                    nc.scalar.copy(sumsT[:, rows], psall[:, rows])
                else:
                    nc.vector.tensor_scalar_mul(sumsT[:, rows],
                                                psall[:, rows], 1.0)
            ots = []
            for t in range(NT):
                m = min(128, S - 128 * t)
                rows = slice(128 * t, 128 * t + m)
                ps3 = psall[:, 1536 + (t % 4) * 512 : 2048 + (t % 4) * 512]
                ot = p3pool.tile([128, FO], F32, tag="ot")
                if t % 2:
                    nc.tensor.matmul(ps3[:m, :], sumsT[:, rows], wt_sb[:],
                                     start=True, stop=True,
                                     skip_group_check=True)
                    nc.vector.scalar_tensor_tensor(
                        ot[:m, :], ps3[:m, :], rb_sb[:m, t : t + 1], b_sb[:m, :],
                        op0=mybir.AluOpType.mult, op1=mybir.AluOpType.add,
                    )
                else:
                    # bias via rank-1 matmul into the same PSUM group, so the
                    # evac is a single per-partition scale on ACT. The bias
                    # rides pre-scale: add b*deg, then *rb restores b.
                    nc.tensor.matmul(ps3[:m, :], sumsT[:, rows], wt_sb[:],
                                     start=True, stop=False,
                                     skip_group_check=True)
                    nc.tensor.matmul(ps3[:m, :], db_sb[:, 128 * t : 128 * t + m],
                                     b_sb[:], start=False, stop=True,
                                     skip_group_check=True)
                    nc.scalar.mul(ot[:m, :], ps3[:m, :], rb_sb[:m, t : t + 1])
                ots.append((ot, rows, m))
            for t, (ot, rows, m) in enumerate(ots):
                deng = nc.scalar if t % 2 else nc.sync
                deng.dma_start(out[rows, :], ot[:m, :])

    nc.compile()
    return nc


def _shard_inputs(x32, src, dst, W32, b32, n_cores):
    import ml_dtypes

    N, F = x32.shape
    FO = W32.shape[1]
    S = (N + n_cores - 1) // n_cores
    NT = (S + 127) // 128
    R = NT * 128
    KT = (N + 127) // 128

    # x slabs, feature-minor: xt[p, k, f] = x[128k + p, f], bf16
    xp = np.zeros((KT * 128, F), np.float32)
    xp[:N] = x32
    xt = np.ascontiguousarray(
        xp.reshape(KT, 128, F).transpose(1, 0, 2).astype(ml_dtypes.bfloat16)
    ).reshape(128, KT * F)

    deg = np.bincount(dst, minlength=N)
    rb_full = (1.0 / np.maximum(deg, 1)).astype(np.float32)
    zero_nodes = np.where(deg == 0)[0]

    brep = np.ascontiguousarray(
        np.tile(b32.reshape(1, -1), (128, 1)).astype(ml_dtypes.bfloat16))
    Wb = W32.astype(ml_dtypes.bfloat16)

    in_maps = []
    for c in range(n_cores):
        lo = c * S
        hi = min(N, lo + S)
        sel = (dst >= lo) & (dst < hi)
        A = np.zeros((KT * 128, S), np.float32)
        np.add.at(A, (src[sel], dst[sel] - lo), 1.0)
        zn = zero_nodes[(zero_nodes >= lo) & (zero_nodes < hi)]
        if zn.size:  # self-edge: zero-in-degree nodes keep their input
            A[zn, zn - lo] += 1.0
        assert A.max() <= 16, "edge multiplicity too large for fp8e4m3"
        A8 = np.ascontiguousarray(
            A.reshape(KT, 128, S).transpose(1, 0, 2).astype(ml_dtypes.float8_e4m3)
        ).reshape(128, KT * S)
        rb_c = np.ones(R, np.float32)
        rb_c[: hi - lo] = rb_full[lo:hi]
        # db[n] = max(deg,1)/128 so (sums@W + db*128*b) * rb == mean@W + b
        deg_c = np.ones(S, np.float32)
        deg_c[: hi - lo] = np.maximum(deg[lo:hi], 1)
        db_c = np.ascontiguousarray(np.tile(
            (deg_c / 128.0).astype(ml_dtypes.bfloat16).reshape(1, S),
            (128, 1)))
        rb_c = np.ascontiguousarray(rb_c.reshape(NT, 128).T)
        in_maps.append({"xt": xt, "A": A8, "W": Wb, "b": brep, "rb": rb_c,
                        "db": db_c})
    return in_maps, R


def _install_ntff_shim():
    """antenv.axon_hooks shim so trace=True can NTFF-profile in this env."""
    import contextlib
    import ctypes
    import sys
    import types

    if "antenv.axon_hooks" in sys.modules:
        return
    so_path = "/opt/axon/libaxon_pjrt.so"
    try:
        lib = ctypes.CDLL(so_path)
        lib.axon_start_nrt_profile.argtypes = [
            ctypes.POINTER(ctypes.c_int64), ctypes.c_size_t]
        lib.axon_start_nrt_profile.restype = ctypes.c_int64
        lib.axon_stop_nrt_profile.argtypes = [ctypes.c_char_p]
        lib.axon_stop_nrt_profile.restype = ctypes.c_int64
    except Exception:
        return

    @contextlib.contextmanager
    def _hook(output_dir, device_ids):
        import jax

        jax.devices()
        if device_ids:
            ids = (ctypes.c_int64 * len(device_ids))(*device_ids)
            rc = lib.axon_start_nrt_profile(ids, len(device_ids))
        else:
            rc = lib.axon_start_nrt_profile(None, 0)
        if rc != 0:
            raise RuntimeError(f"axon_start_nrt_profile rc={rc}")
        try:
            yield
        finally:
            lib.axon_stop_nrt_profile(str(output_dir).encode())

    mod = types.ModuleType("antenv.axon_hooks")
    mod.set_axon_ntff_profile_hook = lambda h: None
    mod.get_axon_ntff_profile_hook = lambda: _hook
    sys.modules["antenv.axon_hooks"] = mod


def kernel(x, src, dst, W, b):
    from concourse import bass_utils

    x32 = np.ascontiguousarray(np.asarray(x), dtype=np.float32)
    W32 = np.ascontiguousarray(np.asarray(W), dtype=np.float32)
    b32 = np.ascontiguousarray(np.asarray(b), dtype=np.float32)
    src = np.asarray(src).astype(np.int64)
    dst = np.asarray(dst).astype(np.int64)
    N, F = x32.shape
    FO = W32.shape[1]
    S = (N + CORES - 1) // CORES

    in_maps, R = _shard_inputs(x32, src, dst, W32, b32, CORES)

    key = (N, F, FO, R)
    if key not in _cache:
        _cache[key] = _build_program(N, F, FO, R, S)
    nc = _cache[key]

    if TRACE:
        _install_ntff_shim()

    last_err = None
    for _attempt in range(2):
        try:
            res = bass_utils.run_bass_kernel_spmd(
                nc, in_maps, core_ids=list(range(CORES)), trace=TRACE
            )
            break
        except Exception as e:  # retry once on transient device errors
            last_err = e
    else:
        raise last_err

    if TRACE and res.exec_time_ns is not None:
        print("HW exec time:", res.exec_time_ns, "ns")

    outs = [np.asarray(r["out"]).reshape(R, FO) for r in res.results]
    full = np.concatenate([o[:S] for o in outs], axis=0)[:N]
    return full.astype(np.float32)


# revision 26
# speedup vs baseline: 1.0679x; 1.0015x over previous
"""GCN layer (gather -> mean-aggregate -> linear) on 8 Trainium2 cores.

Strategy (SPMD, no collectives):
  - Nodes are row-sharded: core c owns dst nodes [c*S, (c+1)*S), S = N/8.
  - Edges are bucketed by dst-owner core on the host and turned into a dense
    per-core adjacency count matrix A[src_node, local_dst] (fp8e4m3 - counts
    are small integers, exact). The per-core segment sum is computed
    FEATURE-MAJOR on the PE array:  sumsT = x.T @ A, with x as bf16 slabs
    (lhsT stationary) and A streaming as the rhs.  This streams S columns
    per src slab (vs 2*F+1 per slab per node-group for the node-major
    orientation) - half the PE cycles of the hi/lo node-major scheme.
  - Degrees (and the zero-in-degree fallback) are folded in on the host:
    rb = 1/max(deg,1) is shipped per node, and zero-degree nodes get a
    self-edge in A so mean==x for them (DGL recv semantics), making the
    device program branch-free.
  - Phase 1 runs in two column passes (A-left cols 0:1024, A-right 1024:S,
    shipped as separate streams).  Pass A's eight node tiles drain under
    the pass-B matmul stream: per tile, evac PSUM->SBUF bf16 (sumsT is
    already the GEMM's lhsT layout - no PE transposes), GEMM + rank-1
    bias matmul into PSUM, a single per-partition scale to SBUF
    (DVE / ACT alternating), and the output DMA.
  - Input DMA chunks alternate between the two HWDGE queues in
    consumption order, with tiny head chunks so the PE starts early and
    the HAM clock never re-throttles.
"""

import os

import numpy as np

CORES = 8
TRACE = False           # set by test harness to print HW exec time
_cache = {}


def _build_program(N, F, FO, R, S):
    from concourse import bacc, tile
    from concourse.bass import mybir

    F32 = mybir.dt.float32
    BF16 = mybir.dt.bfloat16
    FP8 = mybir.dt.float8e4
    KT = (N + 127) // 128          # src-node slabs
    NT = (S + 127) // 128          # owned-node tiles per core
    CL = 1024                      # pass-A columns (8 node tiles)
    CR = S - CL                    # pass-B columns
    nc = bacc.Bacc(None)

    xtd = nc.dram_tensor("xt", [128, KT * F], BF16, kind="ExternalInput")
    Ald = nc.dram_tensor("Al", [128, KT * CL], FP8, kind="ExternalInput")
    Ard = nc.dram_tensor("Ar", [128, KT * CR], FP8, kind="ExternalInput")
    Wd = nc.dram_tensor("W", [F, FO], BF16, kind="ExternalInput")
    bd = nc.dram_tensor("b", [128, FO], BF16, kind="ExternalInput")
    rbd = nc.dram_tensor("rb", [128, NT], F32, kind="ExternalInput")
    dbd = nc.dram_tensor("db", [128, S], BF16, kind="ExternalInput")
    out = nc.dram_tensor("out", [R, FO], F32, kind="ExternalOutput")

    # PSUM bank map (each accumulating group owns a 2KB bank, zeroed by
    # its first start=True matmul):
    #   banks 0,1: pass-A col groups [0:512],[512:1024]
    #   bank  2:   pass-B col group [1024:S]
    #   banks 3..6: drain out psum, cycling t%4
    #   bank  7 head: PE warm-up scratch
    psall = nc.alloc_psum_tensor("psall", [128, 4096], F32)

    with tile.TileContext(nc) as tc:
        with (
            tc.tile_pool(name="const", bufs=1) as cpool,
            tc.tile_pool(name="acc", bufs=1) as accpool,
            tc.tile_pool(name="p3", bufs=10) as p3pool,
        ):
            x_sb = accpool.tile([128, KT, F], BF16, name="x_sb", tag="x_sb")
            x_flat = x_sb[:].rearrange("p a b -> p (a b)")
            Al_sb = accpool.tile([128, KT, CL], FP8, name="Al_sb", tag="Al_sb")
            Al_flat = Al_sb[:].rearrange("p a b -> p (a b)")
            Ar_sb = accpool.tile([128, KT, CR], FP8, name="Ar_sb", tag="Ar_sb")
            Ar_flat = Ar_sb[:].rearrange("p a b -> p (a b)")
            sumsT = accpool.tile([128, S], BF16, name="sumsT")
            wt_sb = cpool.tile([128, FO], BF16, name="wt_sb")
            b_sb = cpool.tile([128, FO], BF16, name="b_sb")
            rb_sb = cpool.tile([128, NT], F32, name="rb_sb")
            db_sb = cpool.tile([128, S], BF16, name="db_sb")

            # ---- all input DMA up front: chunks in consumption order,
            # alternating HWDGE queues so each ring's backlog is ordered
            # and either ring alone can nearly feed the PE ----
            xfers = []   # (order, kind, k0, k1)
            bal = [0, 1, 2, 4, 6, 9, 13, 17, 22, 27, 33, 39, 45, 51, 57,
                   63, 71, KT]
            for i, (k0, k1) in enumerate(zip(bal, bal[1:])):
                xfers.append(((k0, 1), "Al", k0, k1))
            bx = [0, 1, 2, 4, 8, 16, 32, 56, KT]
            for k0, k1 in zip(bx, bx[1:]):
                xfers.append(((k0, 0), "x", k0, k1))
            xfers.sort(key=lambda t: t[0])
            qi = 0
            for _, kind, k0, k1 in xfers:
                deng = nc.sync if qi % 2 == 0 else nc.scalar
                qi += 1
                if kind == "Al":
                    deng.dma_start(Al_flat[:, k0 * CL : k1 * CL],
                                   Ald[:, k0 * CL : k1 * CL])
                else:
                    deng.dma_start(x_flat[:, k0 * F : k1 * F],
                                   xtd[:, k0 * F : k1 * F])
                if qi == 5:  # constants early but off the queue heads
                    nc.scalar.dma_start(wt_sb[:], Wd[:])
                    nc.scalar.dma_start(b_sb[:], bd[:])
                    nc.scalar.dma_start(rb_sb[:], rbd[:])
                    nc.scalar.dma_start(db_sb[:], dbd[:])
            for j, (k0, k1) in enumerate(zip([0, 26, 52, KT], [26, 52, KT])):
                deng = nc.sync if j % 2 == 0 else nc.scalar
                deng.dma_start(Ar_flat[:, k0 * CR : k1 * CR],
                               Ard[:, k0 * CR : k1 * CR])

            # PE warm-up: tiny matmuls during the first-chunk DMA wait so
            # the HAM clock gate is at full rate for the real stream.
            warm = cpool.tile([128, 128], BF16, name="warm")
            nc.vector.memset(warm[:], 0.0)
            for _w in range(20):
                nc.tensor.matmul(
                    psall[:16, 3584:3712], warm[:, 0:16], warm[:, 0:128],
                    start=True, stop=True, skip_group_check=True,
                )

            # ---- phase 1 pass A: sumsT[f, d] += x[s, f] * A[s, d] ----
            for k in range(KT):
                st = k == 0
                sp = k == KT - 1
                for (c0, c1) in [(0, 512), (512, 1024)]:
                    nc.tensor.matmul(
                        psall[:, c0:c1], x_sb[:, k, :], Al_sb[:, k, c0:c1],
                        start=st, stop=sp, skip_group_check=False,
                    )

            # evacuate pass-A tiles (PSUM -> SBUF bf16), alternating engines
            for t in range(8):
                cs = slice(128 * t, 128 * (t + 1))
                if t % 2:
                    nc.scalar.copy(sumsT[:, cs], psall[:, cs])
                else:
                    nc.vector.tensor_scalar_mul(sumsT[:, cs], psall[:, cs], 1.0)

            # drain one node tile: GEMM (+rank-1 bias), scale, store
            ots = []

            def drain_tile(t):
                m = min(128, S - 128 * t)      # last tile is a remnant
                rows = slice(128 * t, 128 * t + m)
                ps3 = psall[:, 1536 + (t % 4) * 512 : 2048 + (t % 4) * 512]
                ot = p3pool.tile([128, FO], F32, tag="ot")
                if t % 2:
                    nc.tensor.matmul(ps3[:m, :], sumsT[:, rows], wt_sb[:],
                                     start=True, stop=True,
                                     skip_group_check=True)
                    nc.vector.scalar_tensor_tensor(
                        ot[:m, :], ps3[:m, :], rb_sb[:m, t : t + 1], b_sb[:m, :],
                        op0=mybir.AluOpType.mult, op1=mybir.AluOpType.add,
                    )
                else:
                    # bias via rank-1 matmul into the same PSUM group: add
                    # b*deg pre-scale, the *rb evac restores b.
                    nc.tensor.matmul(ps3[:m, :], sumsT[:, rows], wt_sb[:],
                                     start=True, stop=False,
                                     skip_group_check=True)
                    nc.tensor.matmul(ps3[:m, :], db_sb[:, 128 * t : 128 * t + m],
                                     b_sb[:], start=False, stop=True,
                                     skip_group_check=True)
                    nc.scalar.mul(ot[:m, :], ps3[:m, :], rb_sb[:m, t : t + 1])
                ots.append((ot, rows, m))

            # ---- phase 1 pass B, pass-A tiles draining underneath ----
            nxt_drain = 0
            for k in range(KT):
                st = k == 0
                sp = k == KT - 1
                nc.tensor.matmul(
                    psall[:, 1024 : 1024 + CR], x_sb[:, k, :], Ar_sb[:, k, :],
                    start=st, stop=sp, skip_group_check=False,
                )
                if k % 9 == 8 and nxt_drain < 8:
                    drain_tile(nxt_drain)
                    nxt_drain += 1

            # evacuate + drain the pass-B tiles
            for t in range(8, NT):
                cs = slice(128 * t, min(128 * (t + 1), S))
                if t % 2:
                    nc.scalar.copy(sumsT[:, cs], psall[:, cs])
                else:
                    nc.vector.tensor_scalar_mul(sumsT[:, cs], psall[:, cs], 1.0)
            while nxt_drain < NT:
                drain_tile(nxt_drain)
                nxt_drain += 1

            for t, (ot, rows, m) in enumerate(ots):
                deng = nc.scalar if t % 2 else nc.sync
                deng.dma_start(out[rows, :], ot[:m, :])

    nc.compile()
    return nc


def _shard_inputs(x32, src, dst, W32, b32, n_cores):
    import ml_dtypes

    N, F = x32.shape
    FO = W32.shape[1]
    S = (N + n_cores - 1) // n_cores
    NT = (S + 127) // 128
    R = NT * 128
    KT = (N + 127) // 128
    CL = 1024

    # x slabs, feature-minor: xt[p, k, f] = x[128k + p, f], bf16
    xp = np.zeros((KT * 128, F), np.float32)
    xp[:N] = x32
    xt = np.ascontiguousarray(
        xp.reshape(KT, 128, F).transpose(1, 0, 2).astype(ml_dtypes.bfloat16)
    ).reshape(128, KT * F)

    deg = np.bincount(dst, minlength=N)
    rb_full = (1.0 / np.maximum(deg, 1)).astype(np.float32)
    zero_nodes = np.where(deg == 0)[0]

    brep = np.ascontiguousarray(
        np.tile(b32.reshape(1, -1), (128, 1)).astype(ml_dtypes.bfloat16))
    Wb = W32.astype(ml_dtypes.bfloat16)

    in_maps = []
    for c in range(n_cores):
        lo = c * S
        hi = min(N, lo + S)
        sel = (dst >= lo) & (dst < hi)
        A = np.zeros((KT * 128, S), np.float32)
        np.add.at(A, (src[sel], dst[sel] - lo), 1.0)
        zn = zero_nodes[(zero_nodes >= lo) & (zero_nodes < hi)]
        if zn.size:  # self-edge: zero-in-degree nodes keep their input
            A[zn, zn - lo] += 1.0
        assert A.max() <= 16, "edge multiplicity too large for fp8e4m3"
        A3 = A.reshape(KT, 128, S).transpose(1, 0, 2).astype(ml_dtypes.float8_e4m3)
        Al = np.ascontiguousarray(A3[:, :, :CL]).reshape(128, KT * CL)
        Ar = np.ascontiguousarray(A3[:, :, CL:]).reshape(128, KT * (S - CL))
        rb_c = np.ones(R, np.float32)
        rb_c[: hi - lo] = rb_full[lo:hi]
        # db[n] = max(deg,1)/128 so (sums@W + db*128*b) * rb == mean@W + b
        deg_c = np.ones(S, np.float32)
        deg_c[: hi - lo] = np.maximum(deg[lo:hi], 1)
        db_c = np.ascontiguousarray(np.tile(
            (deg_c / 128.0).astype(ml_dtypes.bfloat16).reshape(1, S),
            (128, 1)))
        rb_c = np.ascontiguousarray(rb_c.reshape(NT, 128).T)
        in_maps.append({"xt": xt, "Al": Al, "Ar": Ar, "W": Wb, "b": brep,
                        "rb": rb_c, "db": db_c})
    return in_maps, R


def _install_ntff_shim():
    """antenv.axon_hooks shim so trace=True can NTFF-profile in this env."""
    import contextlib
    import ctypes
    import sys
    import types

    if "antenv.axon_hooks" in sys.modules:
        return
    so_path = "/opt/axon/libaxon_pjrt.so"
    try:
        lib = ctypes.CDLL(so_path)
        lib.axon_start_nrt_profile.argtypes = [
            ctypes.POINTER(ctypes.c_int64), ctypes.c_size_t]
        lib.axon_start_nrt_profile.restype = ctypes.c_int64
        lib.axon_stop_nrt_profile.argtypes = [ctypes.c_char_p]
        lib.axon_stop_nrt_profile.restype = ctypes.c_int64
    except Exception:
        return

    @contextlib.contextmanager
    def _hook(output_dir, device_ids):
        import jax

        jax.devices()
        if device_ids:
            ids = (ctypes.c_int64 * len(device_ids))(*device_ids)
            rc = lib.axon_start_nrt_profile(ids, len(device_ids))
        else:
            rc = lib.axon_start_nrt_profile(None, 0)
        if rc != 0:
            raise RuntimeError(f"axon_start_nrt_profile rc={rc}")
        try:
            yield
        finally:
            lib.axon_stop_nrt_profile(str(output_dir).encode())

    mod = types.ModuleType("antenv.axon_hooks")
    mod.set_axon_ntff_profile_hook = lambda h: None
    mod.get_axon_ntff_profile_hook = lambda: _hook
    sys.modules["antenv.axon_hooks"] = mod


def kernel(x, src, dst, W, b):
    from concourse import bass_utils

    x32 = np.ascontiguousarray(np.asarray(x), dtype=np.float32)
    W32 = np.ascontiguousarray(np.asarray(W), dtype=np.float32)
    b32 = np.ascontiguousarray(np.asarray(b), dtype=np.float32)
    src = np.asarray(src).astype(np.int64)
    dst = np.asarray(dst).astype(np.int64)
    N, F = x32.shape
    FO = W32.shape[1]
    S = (N + CORES - 1) // CORES

    in_maps, R = _shard_inputs(x32, src, dst, W32, b32, CORES)

    key = (N, F, FO, R)
    if key not in _cache:
        _cache[key] = _build_program(N, F, FO, R, S)
    nc = _cache[key]

    if TRACE:
        _install_ntff_shim()

    last_err = None
    for _attempt in range(2):
        try:
            res = bass_utils.run_bass_kernel_spmd(
                nc, in_maps, core_ids=list(range(CORES)), trace=TRACE
            )
            break
        except Exception as e:  # retry once on transient device errors
            last_err = e
    else:
        raise last_err

    if TRACE and res.exec_time_ns is not None:
        print("HW exec time:", res.exec_time_ns, "ns")

    outs = [np.asarray(r["out"]).reshape(R, FO) for r in res.results]
    full = np.concatenate([o[:S] for o in outs], axis=0)[:N]
    return full.astype(np.float32)
